# revision 2
# baseline (speedup 1.0000x reference)
"""Trainium2 Bass kernel for nn_DifferentialDropout_v2 (histogram_binning).

Strategy (per sharding hint): shard the flattened feature dim D across 8
NeuronCores. Each core computes, over its D-shard:
  - partial Gram matrix G_c = X_c X_c^T  (PE: transpose + matmul)
  - partial row sums s_c                  (DVE fused add-reduce)
  - partial per-row threshold counts      (ACT sign+accum / DVE is_ge+accum)
Host merges the partials (all-reduce over cores), derives corr/mse/entropy
factors -> p[64], then a second elementwise kernel applies the dropout mask
out = (u > p_row) * x / (1 - p[63]) with u reproduced from the reference's
fixed PRNG stream (threefry, key 42).

The batch entropy reproduces the reference's fp32 segment_sum semantics
bit-exactly: counts accumulate in fp32 (saturate at 2^24) and the entropy
terms sum sequentially in fp32.
"""

import os
import sys

sys.path.insert(0, "/opt/trn_rl_repo")

import numpy as np

import concourse.bass as bass  # noqa: E402
import concourse.bacc as bacc  # noqa: E402
import concourse.mybir as mybir  # noqa: E402
from concourse import tile, masks  # noqa: E402
from concourse.bass_utils import run_bass_kernel_spmd  # noqa: E402

F32 = mybir.dt.float32
AF = mybir.ActivationFunctionType
OP = mybir.AluOpType

B = 64
D = 802816  # 256*56*56
NCORES = 8
DSH = D // NCORES  # 100352
F = DSH // 2  # 50176 columns in the half-stacked [128, F] layout
CHUNK = 3584  # F = 14 * 3584;  3584 = 28 * 128
NTILE = F // CHUNK  # 14
NBLK = CHUNK // 128  # 28 transpose blocks per chunk
NTHRESH = 12  # thresholds -5.5 .. 5.5 -> bins -6..6
NSLOT = NTHRESH + 1  # + row-sum slot
THRESH = [-5.5 + k for k in range(NTHRESH)]

TRACE = os.environ.get("BASS_PROBLEM_TRACE", "") == "1"
LAST_EXEC_NS = []

_cache = {}


def _maybe_register_trace_hook():
    if not TRACE:
        return
    try:
        from antenv import axon_hooks
        if axon_hooks.get_axon_ntff_profile_hook() is None:
            from trn_agent_boot.trn_boot import _ntff_profile_via_ctypes
            axon_hooks.set_axon_ntff_profile_hook(
                _ntff_profile_via_ctypes("/opt/axon/libaxon_pjrt.so")
            )
    except Exception:
        pass


def _dma_stacked_load(nc, tile_ap, dram_ap, off, length):
    # stacked [128, length] <- [64, DSH]: partition p = h*64 + r
    nc.sync.dma_start(tile_ap[0:64, :], dram_ap[:, off : off + length])
    nc.sync.dma_start(tile_ap[64:128, :], dram_ap[:, F + off : F + off + length])


def _dma_stacked_store(nc, dram_ap, tile_ap, off, length):
    nc.sync.dma_start(dram_ap[:, off : off + length], tile_ap[0:64, :])
    nc.sync.dma_start(dram_ap[:, F + off : F + off + length], tile_ap[64:128, :])


def _build_stats():
    nc = bacc.Bacc(None, target_bir_lowering=False, debug=False)
    x_in = nc.declare_dram_parameter("xs", [B, DSH], F32, isOutput=False)
    g_out = nc.declare_dram_parameter("gs", [B, B], F32, isOutput=True)
    h_out = nc.declare_dram_parameter("hs", [128, NSLOT], F32, isOutput=True)

    with tile.TileContext(nc) as tc:
        with (
            tc.tile_pool(name="xp", bufs=3) as xp,
            tc.tile_pool(name="tp", bufs=3) as tp,
            tc.tile_pool(name="cp", bufs=1) as cp,
            tc.tile_pool(name="ps", bufs=3, space="PSUM") as ps,
            tc.tile_pool(name="gp", bufs=2, space="PSUM") as gp,
        ):
            ident = cp.tile([128, 128], F32)
            masks.make_identity(nc, ident[:])
            acc = cp.tile([128, NSLOT * NTILE], F32)
            trash_a = cp.tile([128, CHUNK], F32)
            trash_d = cp.tile([128, CHUNK], F32)
            g_acc = cp.tile([B, B], F32)
            nc.vector.memset(g_acc[:], 0.0)
            biases = cp.tile([128, NTHRESH], F32)
            for k, t in enumerate(THRESH):
                nc.vector.memset(biases[:, k : k + 1], -t)

            for it in range(NTILE):
                xt = xp.tile([128, CHUNK], F32)
                _dma_stacked_load(nc, xt, x_in[:], it * CHUNK, CHUNK)

                # threshold counts: even slots on ACT (sum of sign(x-t)),
                # odd slots on DVE (count of x >= t)
                for k in range(NTHRESH):
                    slot = acc[:, k * NTILE + it : k * NTILE + it + 1]
                    if k % 2 == 0:
                        nc.scalar.activation(
                            trash_a[:], xt[:], AF.Sign,
                            bias=biases[:, k : k + 1], accum_out=slot,
                        )
                    else:
                        nc.vector.tensor_scalar(
                            out=trash_d[:], in0=xt[:], scalar1=THRESH[k],
                            scalar2=None, op0=OP.is_ge, op1=OP.add,
                            accum_out=slot,
                        )
                # row sums (DVE fused reduce)
                nc.vector.tensor_scalar(
                    out=trash_d[:], in0=xt[:], scalar1=0.0, scalar2=None,
                    op0=OP.add, op1=OP.add,
                    accum_out=acc[:, NTHRESH * NTILE + it : NTHRESH * NTILE + it + 1],
                )

                # Gram: transpose 128-blocks via PE, then accumulate
                # G += T_half^T @ T_half for both stacked halves
                g_ps = gp.tile([B, B], F32)
                nmm = 0
                for grp in range(NBLK // 4):
                    stage = ps.tile([128, 512], F32)
                    for b in range(4):
                        blk = grp * 4 + b
                        nc.tensor.transpose(
                            stage[:, b * 128 : (b + 1) * 128],
                            xt[:, blk * 128 : (blk + 1) * 128],
                            ident[:],
                        )
                    tsb = tp.tile([128, 512], F32)
                    nc.vector.tensor_copy(tsb[:], stage[:])
                    for b in range(4):
                        tb = tsb[:, b * 128 : (b + 1) * 128]
                        nc.tensor.matmul(
                            g_ps[:], tb[:, 0:64], tb[:, 0:64],
                            start=(nmm == 0), stop=False,
                        )
                        nmm += 1
                        nc.tensor.matmul(
                            g_ps[:], tb[:, 64:128], tb[:, 64:128],
                            start=False, stop=(nmm == 2 * NBLK - 1),
                        )
                        nmm += 1
                nc.vector.tensor_tensor(
                    out=g_acc[:], in0=g_acc[:], in1=g_ps[:], op=OP.add
                )

            hs = cp.tile([128, NSLOT], F32)
            nc.vector.tensor_reduce(
                hs[:],
                acc[:].rearrange("p (k t) -> p k t", t=NTILE),
                axis=mybir.AxisListType.X,
                op=OP.add,
            )
            nc.sync.dma_start(h_out[:], hs[:])
            nc.sync.dma_start(g_out[:], g_acc[:])

    nc.compile()
    return nc


def _build_apply():
    nc = bacc.Bacc(None, target_bir_lowering=False, debug=False)
    x_in = nc.declare_dram_parameter("xs", [B, DSH], F32, isOutput=False)
    u_in = nc.declare_dram_parameter("us", [B, DSH], F32, isOutput=False)
    np_in = nc.declare_dram_parameter("negp", [128, 1], F32, isOutput=False)
    sc_in = nc.declare_dram_parameter("scale", [128, 1], F32, isOutput=False)
    o_out = nc.declare_dram_parameter("out", [B, DSH], F32, isOutput=True)

    ACH = 3584
    ANT = F // ACH

    with tile.TileContext(nc) as tc:
        with (
            tc.tile_pool(name="xp", bufs=2) as xp,
            tc.tile_pool(name="up", bufs=2) as up,
            tc.tile_pool(name="mp", bufs=2) as mp,
            tc.tile_pool(name="op_", bufs=2) as op_,
            tc.tile_pool(name="cp", bufs=1) as cp,
        ):
            negp = cp.tile([128, 1], F32)
            scale = cp.tile([128, 1], F32)
            nc.sync.dma_start(negp[:], np_in[:])
            nc.sync.dma_start(scale[:], sc_in[:])

            for it in range(ANT):
                xt = xp.tile([128, ACH], F32)
                ut = up.tile([128, ACH], F32)
                _dma_stacked_load(nc, xt, x_in[:], it * ACH, ACH)
                _dma_stacked_load(nc, ut, u_in[:], it * ACH, ACH)
                m = mp.tile([128, ACH], F32)
                # m = sign(u - p) -> relu -> {0,1}; sign(0)=0 matches u>p
                nc.scalar.activation(m[:], ut[:], AF.Sign, bias=negp[:])
                nc.scalar.activation(m[:], m[:], AF.Relu)
                # x * inv  (per-partition scalar), then mask
                nc.vector.tensor_scalar(
                    out=xt[:], in0=xt[:], scalar1=scale[:], scalar2=None,
                    op0=OP.mult,
                )
                ot = op_.tile([128, ACH], F32)
                nc.vector.tensor_tensor(
                    out=ot[:], in0=m[:], in1=xt[:], op=OP.mult
                )
                _dma_stacked_store(nc, o_out[:], ot, it * ACH, ACH)

    nc.compile()
    return nc


def _get_kernels():
    if "stats" not in _cache:
        _cache["stats"] = _build_stats()
    if "apply" not in _cache:
        _cache["apply"] = _build_apply()
    return _cache["stats"], _cache["apply"]


def _run(nc, in_maps):
    _maybe_register_trace_hook()
    res = run_bass_kernel_spmd(nc, in_maps, list(range(NCORES)), trace=TRACE)
    if res.exec_time_ns is not None:
        LAST_EXEC_NS.append(res.exec_time_ns)
    return res.results


def _entropy_fp32(counts_int):
    """Entropy (base2) replicating the reference's fp32 semantics.

    counts_int: exact integer counts in ascending bin-value order.
    fp32 ones-accumulation saturates at 2^24; terms summed sequentially
    in fp32 (zero-count bins excluded, as in the reference).
    """
    c = counts_int[counts_int > 0].astype(np.float64)
    c = np.minimum(c, float(2 ** 24)).astype(np.float32)
    n = np.float32(float(counts_int.sum()))
    p = (c / n).astype(np.float32)
    t = (-(p * np.log2(p).astype(np.float32))).astype(np.float32)
    h = np.float32(0.0)
    for v in t:
        h = np.float32(h + v)
    return h


def _host_p(gs_list, hs_list):
    G = np.zeros((B, B), dtype=np.float64)
    cnt_lt = np.zeros((NTHRESH, 128), dtype=np.float64)  # per stacked row
    s_stacked = np.zeros(128, dtype=np.float64)
    for gs, hs in zip(gs_list, hs_list):
        G += gs.astype(np.float64)
        h = hs.astype(np.float64)
        for k in range(NTHRESH):
            if k % 2 == 0:  # ACT: sum of sign -> #lt = (N - S)/2 (no ties)
                cnt_lt[k] += (F - h[:, k]) / 2.0
            else:  # DVE: count of x >= t -> #lt = N - count
                cnt_lt[k] += F - h[:, k]
        s_stacked += h[:, NTHRESH]
    s = s_stacked[:B] + s_stacked[B:]
    A = cnt_lt[:, :B] + cnt_lt[:, B:]  # [12, 64]  #x < t per full row

    # bins -6..6 (13): c_v = A_{v+0.5} - A_{v-0.5}
    row_hist = np.zeros((B, NTHRESH + 1), dtype=np.int64)
    Ar = np.rint(A).astype(np.int64)
    row_hist[:, 0] = Ar[0]
    for k in range(1, NTHRESH):
        row_hist[:, k] = Ar[k] - Ar[k - 1]
    row_hist[:, NTHRESH] = D - Ar[NTHRESH - 1]

    row_ents = np.array(
        [_entropy_fp32(row_hist[i]) for i in range(B)], dtype=np.float64
    )
    batch_ent = float(_entropy_fp32(row_hist.sum(axis=0)))

    cov = G - np.outer(s, s) / D
    dg = np.diag(cov)
    corr = cov / np.sqrt(np.outer(dg, dg))
    factor1 = np.abs(corr).mean(axis=1)

    gdiag = np.diag(G)
    grow = G.sum(axis=1)
    gtot = G.sum()
    row_mse = (gdiag - 2.0 / B * grow + gtot / (B * B)) / D
    factor2 = row_mse / row_mse.sum()

    ratio = row_ents / batch_ent
    factor3 = np.minimum(ratio, 1.0 / ratio)

    p = ((1.0 - factor1) * factor2 * factor3).astype(np.float32)
    return p


def _uniform_u():
    import jax

    cpu = jax.devices("cpu")[0]
    with jax.default_device(cpu):
        key = jax.random.key(42)
        u = jax.random.uniform(key, (B, 256, 56, 56), dtype=np.float32)
        return np.asarray(u).reshape(B, D)


def kernel(x, module=None):
    del module
    LAST_EXEC_NS.clear()
    x = np.asarray(x, dtype=np.float32)
    orig_shape = x.shape
    temp = np.ascontiguousarray(x.reshape(B, D))

    stats_nc, apply_nc = _get_kernels()

    shards = [
        np.ascontiguousarray(temp[:, c * DSH : (c + 1) * DSH])
        for c in range(NCORES)
    ]
    res = _run(stats_nc, [{"xs": s} for s in shards])
    p = _host_p([r["gs"] for r in res], [r["hs"] for r in res])

    u = _uniform_u()
    inv = np.float32(1.0) / (np.float32(1.0) - p[B - 1])
    negp = np.tile(-p, 2).reshape(128, 1).astype(np.float32)
    scale = np.full((128, 1), inv, dtype=np.float32)

    in_maps = []
    for c in range(NCORES):
        in_maps.append(
            {
                "xs": shards[c],
                "us": np.ascontiguousarray(u[:, c * DSH : (c + 1) * DSH]),
                "negp": negp,
                "scale": scale,
            }
        )
    res = _run(apply_nc, in_maps)
    out = np.concatenate([r["out"] for r in res], axis=1)
    return out.reshape(orig_shape)
